# revision 1
# baseline (speedup 1.0000x reference)
"""CrossViewContrastiveLoss Trainium2 kernel.

loss = f(v1^T @ v2) where v1, v2 are [131072, 256] fp32 and f is a cheap
normalize/log epilogue on the [256, 256] joint matrix.

Strategy (data-parallel over N across 8 cores):
  - core c computes partial_c = v1[c*16384:(c+1)*16384]^T @ v2[same rows]
    as a PE GEMM streaming 32 MiB of HBM per core (memory-bound).
  - host sums the eight 256x256 partials in float64 and runs the epilogue
    (65536 elements -- negligible next to 256 MiB of streaming).
"""

import os

import numpy as np

import concourse.bacc as bacc
import concourse.bass as bass
import concourse.mybir as mybir
import concourse.tile as tile
from concourse import bass_utils

N_FULL = 131072
K = 256
NCORES = 8
N_LOC = N_FULL // NCORES  # 16384 rows per core
P = 128
NT = N_LOC // P  # 128 k-tiles of 128 rows per core
CHUNK = int(os.environ.get("CVCL_CHUNK", "8"))  # k-tiles per DMA
ALPHA = 9.0
EPS = 2.220446049250313e-16

# matmul input dtype mode: "bf16", "f32r" (fast fp32 modes), or "f32"
MM_MODE = os.environ.get("CVCL_MM_MODE", "bf16")
# input DMA queue assignment: "split" (v1 sync / v2 gpsimd), "sync", "gpsimd"
DMA_Q = os.environ.get("CVCL_DMA_Q", "split")
# descending chunk sizes at the end to shrink the post-DMA tail
TAIL = os.environ.get("CVCL_TAIL", "0") == "1"

_BUILD_CACHE = {}
LAST_RESULT = None  # BassKernelResults of the most recent run (for test.py)


def _install_axon_hooks_shim():
    """bass_utils' trace path imports antenv.axon_hooks, which this image
    lacks. Provide it, wiring the ctypes NTFF hook from trn_boot when the
    axon .so supports it. Harmless no-op when tracing is off."""
    import sys
    import types

    try:
        from antenv import axon_hooks  # noqa: F401

        return
    except ImportError:
        pass
    try:
        import antenv
    except ImportError:
        return
    mod = types.ModuleType("antenv.axon_hooks")
    mod._hook = None
    mod._resolved = False

    def set_axon_ntff_profile_hook(h):
        mod._hook = h
        mod._resolved = True

    def get_axon_ntff_profile_hook():
        # lazy: only touch the axon .so when tracing is actually requested
        if not mod._resolved:
            mod._resolved = True
            try:
                from trn_agent_boot.trn_boot import _ntff_profile_via_ctypes

                so_path = "/opt/axon/libaxon_pjrt.so"
                if os.path.exists(so_path):
                    mod._hook = _ntff_profile_via_ctypes(so_path)
            except Exception:
                mod._hook = None
        return mod._hook

    mod.set_axon_ntff_profile_hook = set_axon_ntff_profile_hook
    mod.get_axon_ntff_profile_hook = get_axon_ntff_profile_hook
    sys.modules["antenv.axon_hooks"] = mod
    antenv.axon_hooks = mod


try:
    _install_axon_hooks_shim()
except Exception:
    pass


def _build(mode):
    key = (mode, DMA_Q, CHUNK, TAIL)
    if key in _BUILD_CACHE:
        return _BUILD_CACHE[key]

    nc = bacc.Bacc(
        "TRN2", target_bir_lowering=False, debug=False, num_devices=NCORES
    )
    v1 = nc.dram_tensor("v1", [N_LOC, K], mybir.dt.float32, kind="ExternalInput")
    v2 = nc.dram_tensor("v2", [N_LOC, K], mybir.dt.float32, kind="ExternalInput")
    out = nc.dram_tensor("partial", [K, K], mybir.dt.float32, kind="ExternalOutput")

    # [n, k] -> [p, t, k]: k-tile t holds rows t*128 .. t*128+127 on partitions
    v1r = v1.ap().rearrange("(t p) k -> p t k", p=P)
    v2r = v2.ap().rearrange("(t p) k -> p t k", p=P)
    out_ap = out.ap()

    mm_dt = {
        "f32r": mybir.dt.float32r,
        "bf16": mybir.dt.bfloat16,
        "f32": mybir.dt.float32,
    }[mode]

    with tile.TileContext(nc) as tc:
        with (
            tc.tile_pool(name="io", bufs=3) as io_pool,
            tc.tile_pool(name="cv", bufs=3) as cv_pool,
            tc.tile_pool(name="acc", bufs=1, space="PSUM") as psum_pool,
            tc.tile_pool(name="res", bufs=1) as res_pool,
        ):
            # one PSUM bank per 128-row chunk of the [256, 256] output
            ps0 = psum_pool.tile([P, K], mybir.dt.float32)
            ps1 = psum_pool.tile([P, K], mybir.dt.float32)

            # chunk schedule: full-size chunks, then a short descending tail
            # so the last-arriving data has minimal downstream compute
            sizes = []
            rem = NT
            tail = (
                [t for t in (CHUNK // 2, CHUNK // 4, 2, 1, 1) if t >= 1]
                if TAIL
                else []
            )
            tail_total = sum(tail)
            while rem > tail_total:
                sizes.append(CHUNK)
                rem -= CHUNK
            while rem > 0:
                t = next(s for s in tail if s <= rem)
                sizes.append(t)
                rem -= t
            assert sum(sizes) == NT

            q1 = {
                "split": nc.sync,
                "sync": nc.sync,
                "gpsimd": nc.gpsimd,
                "hwsplit": nc.scalar,
            }[DMA_Q]
            q2 = {
                "split": nc.gpsimd,
                "sync": nc.sync,
                "gpsimd": nc.gpsimd,
                "hwsplit": nc.sync,
            }[DMA_Q]

            off = 0
            for ci, csz in enumerate(sizes):
                sl = slice(off, off + csz)
                if mode in ("f32r", "bf16"):
                    # matmul inputs must be rounded by a compute op: DMA raw
                    # fp32 (two queues), then cast v1 on ACT / v2 on DVE.
                    raw1 = io_pool.tile([P, CHUNK, K], mybir.dt.float32, tag="r1")
                    raw2 = io_pool.tile([P, CHUNK, K], mybir.dt.float32, tag="r2")
                    q1.dma_start(raw1[:, 0:csz, :], v1r[:, sl, :])
                    q2.dma_start(raw2[:, 0:csz, :], v2r[:, sl, :])
                    t1 = cv_pool.tile([P, CHUNK, K], mm_dt, tag="c1")
                    t2 = cv_pool.tile([P, CHUNK, K], mm_dt, tag="c2")
                    if DMA_Q == "hwsplit":
                        # scalar queue is issuing DMAs; cast both on DVE
                        nc.vector.tensor_copy(t1[:, 0:csz, :], raw1[:, 0:csz, :])
                    else:
                        nc.scalar.copy(t1[:, 0:csz, :], raw1[:, 0:csz, :])
                    nc.vector.tensor_copy(t2[:, 0:csz, :], raw2[:, 0:csz, :])
                else:
                    t1 = io_pool.tile([P, CHUNK, K], mybir.dt.float32, tag="r1")
                    t2 = io_pool.tile([P, CHUNK, K], mybir.dt.float32, tag="r2")
                    q1.dma_start(t1[:, 0:csz, :], v1r[:, sl, :])
                    q2.dma_start(t2[:, 0:csz, :], v2r[:, sl, :])
                for j in range(csz):
                    first = off + j == 0
                    last = off + j == NT - 1
                    rhs = t2[:, j, :]
                    nc.tensor.matmul(
                        ps0[:],
                        t1[:, j, 0:128],
                        rhs,
                        start=first,
                        stop=last,
                    )
                    nc.tensor.matmul(
                        ps1[:],
                        t1[:, j, 128:256],
                        rhs,
                        start=first,
                        stop=last,
                    )
                off += csz

            res = res_pool.tile([P, 2, K], mybir.dt.float32)
            nc.vector.tensor_copy(res[:, 0, :], ps0[:])
            nc.vector.tensor_copy(res[:, 1, :], ps1[:])
            # out row c*128+p <- res[p, c, :]: one 256 KB DMA
            nc.sync.dma_start(
                out_ap.rearrange("(c p) n -> p c n", p=P), res[:]
            )

    nc.compile()
    _BUILD_CACHE[key] = nc
    return nc


def kernel(latent_view_1, latent_view_2):
    global LAST_RESULT
    v1 = np.ascontiguousarray(np.asarray(latent_view_1, dtype=np.float32))
    v2 = np.ascontiguousarray(np.asarray(latent_view_2, dtype=np.float32))
    assert v1.shape == (N_FULL, K) and v2.shape == (N_FULL, K)

    nc = _build(MM_MODE)
    in_maps = [
        {
            "v1": v1[c * N_LOC : (c + 1) * N_LOC],
            "v2": v2[c * N_LOC : (c + 1) * N_LOC],
        }
        for c in range(NCORES)
    ]
    LAST_RESULT = bass_utils.run_bass_kernel_spmd(
        nc, in_maps, core_ids=list(range(NCORES))
    )

    # host epilogue in float64 on the tiny [256, 256] joint
    p_i_j = np.zeros((K, K), dtype=np.float64)
    for r in LAST_RESULT.results:
        p_i_j += np.asarray(r["partial"], dtype=np.float64)
    p_i_j = (p_i_j + p_i_j.T) / 2.0
    p_i_j = p_i_j / p_i_j.sum()
    p_i = p_i_j.sum(axis=1, keepdims=True)
    p_j = p_i_j.sum(axis=0, keepdims=True)
    p_i_j = np.maximum(p_i_j, EPS)
    p_i = np.maximum(p_i, EPS)
    p_j = np.maximum(p_j, EPS)
    loss = -(
        p_i_j
        * (
            np.log(p_i_j)
            - (ALPHA + 1.0) * np.log(p_j)
            - (ALPHA + 1.0) * np.log(p_i)
        )
    ).sum()
    return np.array(loss, dtype=np.float32)



# revision 2
# speedup vs baseline: 1.0509x; 1.0509x over previous
"""CrossViewContrastiveLoss Trainium2 kernel.

loss = f(v1^T @ v2) where v1, v2 are [131072, 256] fp32 and f is a cheap
normalize/log epilogue on the [256, 256] joint matrix.

Strategy (data-parallel over N across 8 cores):
  - host rearranges each core's [16384, 256] row-shard to partition-major
    [128, 16384*256/128] so every DMA descriptor is a large contiguous
    read (16+ KiB) instead of 1 KiB -- keeps the 16 SDMA engines at the
    ~358 GB/s per-core HBM cap.
  - core c computes partial_c = v1_c^T @ v2_c as PE matmuls accumulating
    into two PSUM banks.  Raw fp32 tiles are bitcast to float32r at the
    matmul (full PE rate at free dim 256) -- no cast stage at all.
  - chunk schedule descends at the end (…,8,4,2,1,1 k-tiles) so almost no
    compute remains after the last byte lands.
  - host sums the eight 256x256 partials in float64 and runs the epilogue
    (65536 elements -- negligible next to 256 MiB of streaming).
"""

import os

import numpy as np

import concourse.bacc as bacc
import concourse.bass as bass
import concourse.mybir as mybir
import concourse.tile as tile
from concourse import bass_utils

N_FULL = 131072
K = 256
NCORES = 8
N_LOC = N_FULL // NCORES  # 16384 rows per core
P = 128
NT = N_LOC // P  # 128 k-tiles of 128 rows per core
CHUNK = int(os.environ.get("CVCL_CHUNK", "16"))  # k-tiles per DMA
ALPHA = 9.0
EPS = 2.220446049250313e-16

# matmul input mode: "f32r" (bitcast, no cast stage) or "bf16" (ACT/DVE cast)
MM_MODE = os.environ.get("CVCL_MM_MODE", "f32r")
# input DMA queue assignment: "hwsplit" (v1 sync / v2 scalar, both HWDGE),
# "split" (v1 sync / v2 gpsimd), "sync" (both on sync)
DMA_Q = os.environ.get("CVCL_DMA_Q", "hwsplit")
# descending chunk sizes at the end to shrink the post-DMA tail
TAIL = os.environ.get("CVCL_TAIL", "1") == "1"

_BUILD_CACHE = {}
LAST_RESULT = None  # BassKernelResults of the most recent run (for test.py)


def _install_axon_hooks_shim():
    """bass_utils' trace path imports antenv.axon_hooks, which this image
    lacks. Provide it, wiring the ctypes NTFF hook from trn_boot when the
    axon .so supports it. Harmless no-op when tracing is off."""
    import sys
    import types

    try:
        from antenv import axon_hooks  # noqa: F401

        return
    except ImportError:
        pass
    try:
        import antenv
    except ImportError:
        return
    mod = types.ModuleType("antenv.axon_hooks")
    mod._hook = None
    mod._resolved = False

    def set_axon_ntff_profile_hook(h):
        mod._hook = h
        mod._resolved = True

    def get_axon_ntff_profile_hook():
        # lazy: only touch the axon .so when tracing is actually requested
        if not mod._resolved:
            mod._resolved = True
            try:
                from trn_agent_boot.trn_boot import _ntff_profile_via_ctypes

                so_path = "/opt/axon/libaxon_pjrt.so"
                if os.path.exists(so_path):
                    mod._hook = _ntff_profile_via_ctypes(so_path)
            except Exception:
                mod._hook = None
        return mod._hook

    mod.set_axon_ntff_profile_hook = set_axon_ntff_profile_hook
    mod.get_axon_ntff_profile_hook = get_axon_ntff_profile_hook
    sys.modules["antenv.axon_hooks"] = mod
    antenv.axon_hooks = mod


try:
    _install_axon_hooks_shim()
except Exception:
    pass


def _chunk_schedule():
    """Chunk sizes (in k-tiles) summing to NT: full-size chunks, then a
    short descending tail so the last-arriving data has minimal
    downstream compute."""
    sizes = []
    rem = NT
    tail = (
        [t for t in (CHUNK // 2, CHUNK // 4, 2, 1, 1) if t >= 1] if TAIL else []
    )
    tail_total = sum(tail)
    while rem > tail_total:
        sizes.append(CHUNK)
        rem -= CHUNK
    while rem > 0:
        t = next(s for s in tail if s <= rem)
        sizes.append(t)
        rem -= t
    assert sum(sizes) == NT
    return sizes


def _build(mode):
    key = (mode, DMA_Q, CHUNK, TAIL)
    if key in _BUILD_CACHE:
        return _BUILD_CACHE[key]

    nc = bacc.Bacc(
        "TRN2", target_bir_lowering=False, debug=False, num_devices=NCORES
    )
    # host-rearranged, partition-major: row t*128+p of the shard lives at
    # v[p, t*K : (t+1)*K] -- each partition's stream is fully contiguous
    v1 = nc.dram_tensor("v1", [P, NT * K], mybir.dt.float32, kind="ExternalInput")
    v2 = nc.dram_tensor("v2", [P, NT * K], mybir.dt.float32, kind="ExternalInput")
    out = nc.dram_tensor("partial", [K, K], mybir.dt.float32, kind="ExternalOutput")

    v1r = v1.ap()
    v2r = v2.ap()
    out_ap = out.ap()

    f32r = mybir.dt.float32r

    with tile.TileContext(nc) as tc:
        with (
            tc.tile_pool(name="io", bufs=3) as io_pool,
            tc.tile_pool(name="cv", bufs=3) as cv_pool,
            tc.tile_pool(name="acc", bufs=1, space="PSUM") as psum_pool,
            tc.tile_pool(name="res", bufs=1) as res_pool,
        ):
            # one PSUM bank per 128-row chunk of the [256, 256] output
            ps0 = psum_pool.tile([P, K], mybir.dt.float32)
            ps1 = psum_pool.tile([P, K], mybir.dt.float32)

            sizes = _chunk_schedule()

            q1 = {
                "hwsplit": nc.sync,
                "split": nc.sync,
                "sync": nc.sync,
            }[DMA_Q]
            q2 = {
                "hwsplit": nc.scalar,
                "split": nc.gpsimd,
                "sync": nc.sync,
            }[DMA_Q]

            off = 0
            for csz in sizes:
                sl = slice(off * K, (off + csz) * K)
                t1 = io_pool.tile([P, CHUNK * K], mybir.dt.float32, tag="r1")
                t2 = io_pool.tile([P, CHUNK * K], mybir.dt.float32, tag="r2")
                q1.dma_start(t1[:, 0 : csz * K], v1r[:, sl])
                q2.dma_start(t2[:, 0 : csz * K], v2r[:, sl])
                if mode == "bf16":
                    c1 = cv_pool.tile(
                        [P, CHUNK * K], mybir.dt.bfloat16, tag="c1"
                    )
                    c2 = cv_pool.tile(
                        [P, CHUNK * K], mybir.dt.bfloat16, tag="c2"
                    )
                    nc.scalar.copy(c1[:, 0 : csz * K], t1[:, 0 : csz * K])
                    nc.vector.tensor_copy(c2[:, 0 : csz * K], t2[:, 0 : csz * K])
                    m1, m2 = c1, c2
                else:
                    m1, m2 = t1, t2
                for j in range(csz):
                    first = off + j == 0
                    last = off + j == NT - 1
                    lhs_a = m1[:, j * K : j * K + 128]
                    lhs_b = m1[:, j * K + 128 : (j + 1) * K]
                    rhs = m2[:, j * K : (j + 1) * K]
                    if mode == "f32r":
                        lhs_a = lhs_a.bitcast(f32r)
                        lhs_b = lhs_b.bitcast(f32r)
                        rhs = rhs.bitcast(f32r)
                    nc.tensor.matmul(
                        ps0[:], lhs_a, rhs, start=first, stop=last
                    )
                    nc.tensor.matmul(
                        ps1[:], lhs_b, rhs, start=first, stop=last
                    )
                off += csz

            res = res_pool.tile([P, 2, K], mybir.dt.float32)
            nc.vector.tensor_copy(res[:, 0, :], ps0[:])
            nc.vector.tensor_copy(res[:, 1, :], ps1[:])
            # out row c*128+p <- res[p, c, :]: one 256 KB DMA
            nc.sync.dma_start(
                out_ap.rearrange("(c p) n -> p c n", p=P), res[:]
            )

    nc.compile()
    _BUILD_CACHE[key] = nc
    return nc


def _rearrange_shards(v):
    """[N_FULL, K] -> [NCORES, P, NT*K]: core c, partition p holds rows
    {t*128+p} of its shard, each partition's stream contiguous."""
    return np.ascontiguousarray(
        v.reshape(NCORES, NT, P, K).transpose(0, 2, 1, 3).reshape(NCORES, P, NT * K)
    )


def kernel(latent_view_1, latent_view_2):
    global LAST_RESULT
    v1 = np.asarray(latent_view_1, dtype=np.float32)
    v2 = np.asarray(latent_view_2, dtype=np.float32)
    assert v1.shape == (N_FULL, K) and v2.shape == (N_FULL, K)

    nc = _build(MM_MODE)
    v1s = _rearrange_shards(v1)
    v2s = _rearrange_shards(v2)
    in_maps = [{"v1": v1s[c], "v2": v2s[c]} for c in range(NCORES)]
    LAST_RESULT = bass_utils.run_bass_kernel_spmd(
        nc, in_maps, core_ids=list(range(NCORES))
    )

    # host epilogue in float64 on the tiny [256, 256] joint
    p_i_j = np.zeros((K, K), dtype=np.float64)
    for r in LAST_RESULT.results:
        p_i_j += np.asarray(r["partial"], dtype=np.float64)
    p_i_j = (p_i_j + p_i_j.T) / 2.0
    p_i_j = p_i_j / p_i_j.sum()
    p_i = p_i_j.sum(axis=1, keepdims=True)
    p_j = p_i_j.sum(axis=0, keepdims=True)
    p_i_j = np.maximum(p_i_j, EPS)
    p_i = np.maximum(p_i, EPS)
    p_j = np.maximum(p_j, EPS)
    loss = -(
        p_i_j
        * (
            np.log(p_i_j)
            - (ALPHA + 1.0) * np.log(p_j)
            - (ALPHA + 1.0) * np.log(p_i)
        )
    ).sum()
    return np.array(loss, dtype=np.float32)


# revision 7
# speedup vs baseline: 1.0716x; 1.0197x over previous
"""CrossViewContrastiveLoss Trainium2 kernel.

loss = f(v1^T @ v2) where v1, v2 are [131072, 256] fp32 and f is a cheap
normalize/log epilogue on the [256, 256] joint matrix.

Strategy (data-parallel over N across 8 cores):
  - host rearranges each core's [16384, 256] row-shard to partition-major
    [128, 16384*256/128] so every DMA descriptor is a large contiguous
    read (16+ KiB) instead of 1 KiB -- keeps the 16 SDMA engines at the
    ~358 GB/s per-core HBM cap.
  - core c computes partial_c = v1_c^T @ v2_c as PE matmuls accumulating
    into two PSUM banks.  Raw fp32 tiles are bitcast to float32r at the
    matmul (full PE rate at free dim 256) -- no cast stage at all.
  - chunk schedule descends at the end (…,8,4,2,1,1 k-tiles) so almost no
    compute remains after the last byte lands.
  - host sums the eight 256x256 partials in float64 and runs the epilogue
    (65536 elements -- negligible next to 256 MiB of streaming).
"""

import os

import numpy as np

import concourse.bacc as bacc
import concourse.bass as bass
import concourse.mybir as mybir
import concourse.tile as tile
from concourse import bass_utils

N_FULL = 131072
K = 256
NCORES = 8
N_LOC = N_FULL // NCORES  # 16384 rows per core
P = 128
NT = N_LOC // P  # 128 k-tiles of 128 rows per core
CHUNK = int(os.environ.get("CVCL_CHUNK", "8"))  # k-tiles per DMA
BUFS = int(os.environ.get("CVCL_BUFS", "6"))  # tile-pool depth
ALPHA = 9.0
EPS = 2.220446049250313e-16

# matmul input mode: "bf16" (ACT/DVE cast) or "dmacast" (SWDGE casts in-flight)
MM_MODE = os.environ.get("CVCL_MM_MODE", "bf16")
# input DMA queue assignment: "hwsplit" (v1 sync / v2 scalar, both HWDGE),
# "split" (v1 sync / v2 gpsimd), "sync" (both on sync)
DMA_Q = os.environ.get("CVCL_DMA_Q", "hwsplit")
# descending chunk sizes at the end to shrink the post-DMA tail
TAIL = os.environ.get("CVCL_TAIL", "1") == "1"

_BUILD_CACHE = {}
LAST_RESULT = None  # BassKernelResults of the most recent run (for test.py)


def _install_axon_hooks_shim():
    """bass_utils' trace path imports antenv.axon_hooks, which this image
    lacks. Provide it, wiring the ctypes NTFF hook from trn_boot when the
    axon .so supports it. Harmless no-op when tracing is off."""
    import sys
    import types

    try:
        from antenv import axon_hooks  # noqa: F401

        return
    except ImportError:
        pass
    try:
        import antenv
    except ImportError:
        return
    mod = types.ModuleType("antenv.axon_hooks")
    mod._hook = None
    mod._resolved = False

    def set_axon_ntff_profile_hook(h):
        mod._hook = h
        mod._resolved = True

    def get_axon_ntff_profile_hook():
        # lazy: only touch the axon .so when tracing is actually requested
        if not mod._resolved:
            mod._resolved = True
            try:
                from trn_agent_boot.trn_boot import _ntff_profile_via_ctypes

                so_path = "/opt/axon/libaxon_pjrt.so"
                if os.path.exists(so_path):
                    mod._hook = _ntff_profile_via_ctypes(so_path)
            except Exception:
                mod._hook = None
        return mod._hook

    mod.set_axon_ntff_profile_hook = set_axon_ntff_profile_hook
    mod.get_axon_ntff_profile_hook = get_axon_ntff_profile_hook
    sys.modules["antenv.axon_hooks"] = mod
    antenv.axon_hooks = mod


try:
    _install_axon_hooks_shim()
except Exception:
    pass


def _chunk_schedule():
    """Chunk sizes (in k-tiles) summing to NT.  Small head chunks start
    the PE early (HAM warmup + pipeline prime); a short descending tail
    leaves minimal compute after the last byte lands; CHUNK-sized chunks
    fill the middle."""
    if not TAIL:
        assert NT % CHUNK == 0
        return [CHUNK] * (NT // CHUNK)
    head = [s for s in (1, 2, 4) if s < CHUNK]
    tail = [s for s in (4, 2, 1, 1) if s < CHUNK]
    rem = NT - sum(head) - sum(tail)
    assert rem > 0
    mid = [CHUNK] * (rem // CHUNK)
    if rem % CHUNK:
        mid = [rem % CHUNK] + mid
    sizes = head + mid + tail
    assert sum(sizes) == NT
    return sizes


def _build(mode):
    key = (mode, DMA_Q, CHUNK, TAIL)
    if key in _BUILD_CACHE:
        return _BUILD_CACHE[key]

    nc = bacc.Bacc(
        "TRN2", target_bir_lowering=False, debug=False, num_devices=NCORES
    )
    # host-rearranged, partition-major: row t*128+p of the shard lives at
    # v[p, t*K : (t+1)*K] -- each partition's stream is fully contiguous
    v1 = nc.dram_tensor("v1", [P, NT * K], mybir.dt.float32, kind="ExternalInput")
    v2 = nc.dram_tensor("v2", [P, NT * K], mybir.dt.float32, kind="ExternalInput")
    out = nc.dram_tensor("partial", [K, K], mybir.dt.float32, kind="ExternalOutput")

    v1r = v1.ap()
    v2r = v2.ap()
    out_ap = out.ap()

    f32r = mybir.dt.float32r

    with tile.TileContext(nc) as tc:
        with (
            tc.tile_pool(name="io", bufs=BUFS) as io_pool,
            tc.tile_pool(name="cv", bufs=BUFS) as cv_pool,
            tc.tile_pool(name="acc", bufs=1, space="PSUM") as psum_pool,
            tc.tile_pool(name="res", bufs=1) as res_pool,
        ):
            # one PSUM bank per 128-row chunk of the [256, 256] output
            ps0 = psum_pool.tile([P, K], mybir.dt.float32)
            ps1 = psum_pool.tile([P, K], mybir.dt.float32)

            sizes = _chunk_schedule()

            q1 = {
                "hwsplit": nc.sync,
                "split": nc.sync,
                "sync": nc.sync,
            }[DMA_Q]
            q2 = {
                "hwsplit": nc.scalar,
                "split": nc.gpsimd,
                "sync": nc.sync,
            }[DMA_Q]

            off = 0
            for csz in sizes:
                sl = slice(off * K, (off + csz) * K)
                if mode == "dmacast":
                    # SWDGE casts fp32 -> bf16 in-flight: no cast stage
                    m1 = cv_pool.tile([P, CHUNK * K], mybir.dt.bfloat16, tag="c1")
                    m2 = cv_pool.tile([P, CHUNK * K], mybir.dt.bfloat16, tag="c2")
                    nc.gpsimd.dma_start(m1[:, 0 : csz * K], v1r[:, sl])
                    nc.gpsimd.dma_start(m2[:, 0 : csz * K], v2r[:, sl])
                else:
                    t1 = io_pool.tile([P, CHUNK * K], mybir.dt.float32, tag="r1")
                    t2 = io_pool.tile([P, CHUNK * K], mybir.dt.float32, tag="r2")
                    q1.dma_start(t1[:, 0 : csz * K], v1r[:, sl])
                    q2.dma_start(t2[:, 0 : csz * K], v2r[:, sl])
                    m1 = cv_pool.tile([P, CHUNK * K], mybir.dt.bfloat16, tag="c1")
                    m2 = cv_pool.tile([P, CHUNK * K], mybir.dt.bfloat16, tag="c2")
                    nc.scalar.copy(m1[:, 0 : csz * K], t1[:, 0 : csz * K])
                    nc.vector.tensor_copy(m2[:, 0 : csz * K], t2[:, 0 : csz * K])
                for j in range(csz):
                    first = off + j == 0
                    last = off + j == NT - 1
                    lhs_a = m1[:, j * K : j * K + 128]
                    lhs_b = m1[:, j * K + 128 : (j + 1) * K]
                    rhs = m2[:, j * K : (j + 1) * K]
                    nc.tensor.matmul(
                        ps0[:], lhs_a, rhs, start=first, stop=last
                    )
                    nc.tensor.matmul(
                        ps1[:], lhs_b, rhs, start=first, stop=last
                    )
                off += csz

            res = res_pool.tile([P, 2, K], mybir.dt.float32)
            # parallel PSUM drains: ACT and DVE may touch different banks
            nc.scalar.copy(res[:, 0, :], ps0[:])
            nc.vector.tensor_copy(res[:, 1, :], ps1[:])
            # out row c*128+p <- res[p, c, :]: one 256 KB DMA
            nc.sync.dma_start(
                out_ap.rearrange("(c p) n -> p c n", p=P), res[:]
            )

    nc.compile()
    _BUILD_CACHE[key] = nc
    return nc


def _rearrange_shards(v):
    """[N_FULL, K] -> [NCORES, P, NT*K]: core c, partition p holds rows
    {t*128+p} of its shard, each partition's stream contiguous."""
    return np.ascontiguousarray(
        v.reshape(NCORES, NT, P, K).transpose(0, 2, 1, 3).reshape(NCORES, P, NT * K)
    )


def kernel(latent_view_1, latent_view_2):
    global LAST_RESULT
    v1 = np.asarray(latent_view_1, dtype=np.float32)
    v2 = np.asarray(latent_view_2, dtype=np.float32)
    assert v1.shape == (N_FULL, K) and v2.shape == (N_FULL, K)

    nc = _build(MM_MODE)
    v1s = _rearrange_shards(v1)
    v2s = _rearrange_shards(v2)
    in_maps = [{"v1": v1s[c], "v2": v2s[c]} for c in range(NCORES)]
    LAST_RESULT = bass_utils.run_bass_kernel_spmd(
        nc, in_maps, core_ids=list(range(NCORES))
    )

    # host epilogue in float64 on the tiny [256, 256] joint
    p_i_j = np.zeros((K, K), dtype=np.float64)
    for r in LAST_RESULT.results:
        p_i_j += np.asarray(r["partial"], dtype=np.float64)
    p_i_j = (p_i_j + p_i_j.T) / 2.0
    p_i_j = p_i_j / p_i_j.sum()
    p_i = p_i_j.sum(axis=1, keepdims=True)
    p_j = p_i_j.sum(axis=0, keepdims=True)
    p_i_j = np.maximum(p_i_j, EPS)
    p_i = np.maximum(p_i, EPS)
    p_j = np.maximum(p_j, EPS)
    loss = -(
        p_i_j
        * (
            np.log(p_i_j)
            - (ALPHA + 1.0) * np.log(p_j)
            - (ALPHA + 1.0) * np.log(p_i)
        )
    ).sum()
    return np.array(loss, dtype=np.float32)


# revision 13
# speedup vs baseline: 1.1543x; 1.0772x over previous
"""CrossViewContrastiveLoss Trainium2 kernel.

loss = f(v1^T @ v2) where v1, v2 are [131072, 256] fp32 and f is a cheap
normalize/log epilogue on the [256, 256] joint matrix.

Strategy (data-parallel over N across 8 cores):
  - host rearranges each core's [16384, 256] row-shard to partition-major
    [128, 16384*256/128] so every DMA descriptor is a large contiguous
    read (16+ KiB) instead of 1 KiB -- keeps the 16 SDMA engines at the
    ~358 GB/s per-core HBM cap.
  - core c computes partial_c = v1_c^T @ v2_c as PE matmuls accumulating
    into two PSUM banks.  Raw fp32 tiles are bitcast to float32r at the
    matmul (full PE rate at free dim 256) -- no cast stage at all.
  - chunk schedule descends at the end (…,8,4,2,1,1 k-tiles) so almost no
    compute remains after the last byte lands.
  - host sums the eight 256x256 partials in float64 and runs the epilogue
    (65536 elements -- negligible next to 256 MiB of streaming).
"""

import os

import numpy as np

import concourse.bacc as bacc
import concourse.bass as bass
import concourse.mybir as mybir
import concourse.tile as tile
from concourse import bass_utils

N_FULL = 131072
K = 256
NCORES = 8
N_LOC = N_FULL // NCORES  # 16384 rows per core
P = 128
# DMA engine 15 (serving partitions {92-95,124-127}) moves bytes ~20%
# slower than the rest on this part; rebalance by packing the last NH*64
# rows of each shard as half-height k-tiles on partitions 0-63 (even
# engines only), so the odd engines carry NT_F=120 k-tiles instead of 128.
NH = int(os.environ.get("CVCL_NH", "16"))  # half k-tiles (64 rows each)
NT_F = (N_LOC - NH * 64) // P  # full k-tiles of 128 rows
N_F = NT_F * P  # rows in full k-tiles
CHUNK = int(os.environ.get("CVCL_CHUNK", "8"))  # k-tiles per DMA
BUFS = int(os.environ.get("CVCL_BUFS", "6"))  # tile-pool depth
ALPHA = 9.0
EPS = 2.220446049250313e-16

# matmul input mode: "bf16" (ACT/DVE cast) or "dmacast" (SWDGE casts in-flight)
MM_MODE = os.environ.get("CVCL_MM_MODE", "bf16")
# input DMA queue assignment: "hwsplit" (v1 sync / v2 scalar, both HWDGE),
# "split" (v1 sync / v2 gpsimd), "sync" (both on sync)
DMA_Q = os.environ.get("CVCL_DMA_Q", "hwsplit")
# descending chunk sizes at the end to shrink the post-DMA tail
TAIL = os.environ.get("CVCL_TAIL", "1") == "1"

_BUILD_CACHE = {}
LAST_RESULT = None  # BassKernelResults of the most recent run (for test.py)


def _install_axon_hooks_shim():
    """bass_utils' trace path imports antenv.axon_hooks, which this image
    lacks. Provide it, wiring the ctypes NTFF hook from trn_boot when the
    axon .so supports it. Harmless no-op when tracing is off."""
    import sys
    import types

    try:
        from antenv import axon_hooks  # noqa: F401

        return
    except ImportError:
        pass
    try:
        import antenv
    except ImportError:
        return
    mod = types.ModuleType("antenv.axon_hooks")
    mod._hook = None
    mod._resolved = False

    def set_axon_ntff_profile_hook(h):
        mod._hook = h
        mod._resolved = True

    def get_axon_ntff_profile_hook():
        # lazy: only touch the axon .so when tracing is actually requested
        if not mod._resolved:
            mod._resolved = True
            try:
                from trn_agent_boot.trn_boot import _ntff_profile_via_ctypes

                so_path = "/opt/axon/libaxon_pjrt.so"
                if os.path.exists(so_path):
                    mod._hook = _ntff_profile_via_ctypes(so_path)
            except Exception:
                mod._hook = None
        return mod._hook

    mod.set_axon_ntff_profile_hook = set_axon_ntff_profile_hook
    mod.get_axon_ntff_profile_hook = get_axon_ntff_profile_hook
    sys.modules["antenv.axon_hooks"] = mod
    antenv.axon_hooks = mod


try:
    _install_axon_hooks_shim()
except Exception:
    pass


def _work_schedule():
    """List of ("f"|"h", k-tile offset, k-tile count) DMA-chunk work items.
    Full-tile sizes: small head chunks start the PE early (HAM warmup +
    pipeline prime); a short descending tail leaves minimal compute after
    the last byte lands; CHUNK-sized chunks fill the middle.  Half-tile
    chunks (even DMA engines only) are interleaved mid-stream so they
    never gate the tail."""
    if TAIL:
        head = [s for s in (1, 2, 4) if s < CHUNK]
        tail = [s for s in (4, 2, 1, 1) if s < CHUNK]
    else:
        head, tail = [], []
    rem = NT_F - sum(head) - sum(tail)
    assert rem > 0
    mid = [CHUNK] * (rem // CHUNK)
    if rem % CHUNK:
        mid = [rem % CHUNK] + mid
    sizes = head + mid + tail
    assert sum(sizes) == NT_F

    items = []
    off = 0
    for csz in sizes:
        items.append(("f", off, csz))
        off += csz
    # interleave half-tile chunks (CHUNK halves each) after ~1/3 of the
    # full chunks
    hoff = 0
    pos = len(sizes) // 3
    while hoff < NH:
        hsz = min(CHUNK, NH - hoff)
        items.insert(pos, ("h", hoff, hsz))
        hoff += hsz
        pos += 2
    return items


def _build(mode):
    key = (mode, DMA_Q, CHUNK, TAIL)
    if key in _BUILD_CACHE:
        return _BUILD_CACHE[key]

    nc = bacc.Bacc(
        "TRN2", target_bir_lowering=False, debug=False, num_devices=NCORES
    )
    # host-rearranged, partition-major: row t*128+p of the shard lives at
    # v[p, t*K : (t+1)*K] -- each partition's stream is fully contiguous
    v1 = nc.dram_tensor("v1", [P, NT_F * K], mybir.dt.float32, kind="ExternalInput")
    v2 = nc.dram_tensor("v2", [P, NT_F * K], mybir.dt.float32, kind="ExternalInput")
    if NH:
        v1h = nc.dram_tensor(
            "v1h", [64, NH * K], mybir.dt.float32, kind="ExternalInput"
        )
        v2h = nc.dram_tensor(
            "v2h", [64, NH * K], mybir.dt.float32, kind="ExternalInput"
        )
    out = nc.dram_tensor("partial", [K, K], mybir.dt.float32, kind="ExternalOutput")

    v1r = v1.ap()
    v2r = v2.ap()
    out_ap = out.ap()

    with tile.TileContext(nc) as tc:
        with (
            tc.tile_pool(name="io", bufs=BUFS) as io_pool,
            tc.tile_pool(name="cv", bufs=BUFS) as cv_pool,
            tc.tile_pool(name="acc", bufs=1, space="PSUM") as psum_pool,
            tc.tile_pool(name="res", bufs=1) as res_pool,
        ):
            # one PSUM bank per 128-row chunk of the [256, 256] output
            ps0 = psum_pool.tile([P, K], mybir.dt.float32)
            ps1 = psum_pool.tile([P, K], mybir.dt.float32)

            items = _work_schedule()
            n_ktiles = NT_F + NH
            q1 = {"hwsplit": nc.sync, "split": nc.sync, "sync": nc.sync}[DMA_Q]
            q2 = {"hwsplit": nc.scalar, "split": nc.gpsimd, "sync": nc.sync}[
                DMA_Q
            ]

            emitted = 0
            for kind, off, csz in items:
                sl = slice(off * K, (off + csz) * K)
                rows = P if kind == "f" else 64
                s1 = v1r[:, sl] if kind == "f" else v1h.ap()[:, sl]
                s2 = v2r[:, sl] if kind == "f" else v2h.ap()[:, sl]
                t1 = io_pool.tile([P, CHUNK * K], mybir.dt.float32, tag="r1")
                t2 = io_pool.tile([P, CHUNK * K], mybir.dt.float32, tag="r2")
                q1.dma_start(t1[0:rows, 0 : csz * K], s1)
                q2.dma_start(t2[0:rows, 0 : csz * K], s2)
                m1 = cv_pool.tile([P, CHUNK * K], mybir.dt.bfloat16, tag="c1")
                m2 = cv_pool.tile([P, CHUNK * K], mybir.dt.bfloat16, tag="c2")
                nc.scalar.copy(m1[0:rows, 0 : csz * K], t1[0:rows, 0 : csz * K])
                nc.vector.tensor_copy(
                    m2[0:rows, 0 : csz * K], t2[0:rows, 0 : csz * K]
                )
                for j in range(csz):
                    first = emitted == 0
                    last = emitted == n_ktiles - 1
                    lhs_a = m1[0:rows, j * K : j * K + 128]
                    lhs_b = m1[0:rows, j * K + 128 : (j + 1) * K]
                    rhs = m2[0:rows, j * K : (j + 1) * K]
                    nc.tensor.matmul(
                        ps0[:], lhs_a, rhs, start=first, stop=last
                    )
                    nc.tensor.matmul(
                        ps1[:], lhs_b, rhs, start=first, stop=last
                    )
                    emitted += 1
            assert emitted == n_ktiles

            res = res_pool.tile([P, 2, K], mybir.dt.float32)
            # parallel PSUM drains: ACT and DVE may touch different banks
            nc.scalar.copy(res[:, 0, :], ps0[:])
            nc.vector.tensor_copy(res[:, 1, :], ps1[:])
            # out row c*128+p <- res[p, c, :]: one 256 KB DMA, issued from
            # the scalar queue (its sequencer just wrote res[:,0,:], so no
            # extra cross-engine semaphore hop before the trigger)
            nc.scalar.dma_start(
                out_ap.rearrange("(c p) n -> p c n", p=P), res[:]
            )

    nc.compile()
    _BUILD_CACHE[key] = nc
    return nc


def _rearrange_shards(v):
    """[N_FULL, K] -> full part [NCORES, P, NT_F*K] + half part
    [NCORES, 64, NH*K]: core c, partition p holds rows {t*128+p} of the
    first N_F rows of its shard; the last NH*64 rows sit on partitions
    0-63 as half-height k-tiles.  Each partition's stream is contiguous."""
    x = v.reshape(NCORES, N_LOC, K)
    full = np.ascontiguousarray(
        x[:, :N_F]
        .reshape(NCORES, NT_F, P, K)
        .transpose(0, 2, 1, 3)
        .reshape(NCORES, P, NT_F * K)
    )
    if not NH:
        return full, None
    half = np.ascontiguousarray(
        x[:, N_F:]
        .reshape(NCORES, NH, 64, K)
        .transpose(0, 2, 1, 3)
        .reshape(NCORES, 64, NH * K)
    )
    return full, half


def kernel(latent_view_1, latent_view_2):
    global LAST_RESULT
    v1 = np.asarray(latent_view_1, dtype=np.float32)
    v2 = np.asarray(latent_view_2, dtype=np.float32)
    assert v1.shape == (N_FULL, K) and v2.shape == (N_FULL, K)

    nc = _build(MM_MODE)
    v1s, v1hs = _rearrange_shards(v1)
    v2s, v2hs = _rearrange_shards(v2)
    in_maps = [{"v1": v1s[c], "v2": v2s[c]} for c in range(NCORES)]
    if NH:
        for c in range(NCORES):
            in_maps[c]["v1h"] = v1hs[c]
            in_maps[c]["v2h"] = v2hs[c]
    LAST_RESULT = bass_utils.run_bass_kernel_spmd(
        nc, in_maps, core_ids=list(range(NCORES))
    )

    # host epilogue in float64 on the tiny [256, 256] joint
    p_i_j = np.zeros((K, K), dtype=np.float64)
    for r in LAST_RESULT.results:
        p_i_j += np.asarray(r["partial"], dtype=np.float64)
    p_i_j = (p_i_j + p_i_j.T) / 2.0
    p_i_j = p_i_j / p_i_j.sum()
    p_i = p_i_j.sum(axis=1, keepdims=True)
    p_j = p_i_j.sum(axis=0, keepdims=True)
    p_i_j = np.maximum(p_i_j, EPS)
    p_i = np.maximum(p_i, EPS)
    p_j = np.maximum(p_j, EPS)
    loss = -(
        p_i_j
        * (
            np.log(p_i_j)
            - (ALPHA + 1.0) * np.log(p_j)
            - (ALPHA + 1.0) * np.log(p_i)
        )
    ).sum()
    return np.array(loss, dtype=np.float32)


# revision 15
# speedup vs baseline: 1.2222x; 1.0588x over previous
"""CrossViewContrastiveLoss Trainium2 kernel.

loss = f(v1^T @ v2) where v1, v2 are [131072, 256] fp32 and f is a cheap
normalize/log epilogue on the [256, 256] joint matrix.

Strategy (data-parallel over N across 8 cores):
  - host rearranges each core's [16384, 256] row-shard to partition-major
    [128, 16384*256/128] so every DMA descriptor is a large contiguous
    read (16+ KiB) instead of 1 KiB -- keeps the 16 SDMA engines at the
    ~358 GB/s per-core HBM cap.
  - core c computes partial_c = v1_c^T @ v2_c as PE matmuls accumulating
    into two PSUM banks.  Raw fp32 tiles are bitcast to float32r at the
    matmul (full PE rate at free dim 256) -- no cast stage at all.
  - chunk schedule descends at the end (…,8,4,2,1,1 k-tiles) so almost no
    compute remains after the last byte lands.
  - host sums the eight 256x256 partials in float64 and runs the epilogue
    (65536 elements -- negligible next to 256 MiB of streaming).
"""

import os

import numpy as np

import concourse.bacc as bacc
import concourse.bass as bass
import concourse.mybir as mybir
import concourse.tile as tile
from concourse import bass_utils

N_FULL = 131072
K = 256
NCORES = 8
N_LOC = N_FULL // NCORES  # 16384 rows per core
P = 128
# DMA engine 15 (serving partitions {92-95,124-127}) moves bytes ~20%
# slower than the rest on this part; rebalance by packing the last NH*64
# rows of each shard as half-height k-tiles on partitions 0-63 (even
# engines only), so the odd engines carry NT_F=120 k-tiles instead of 128.
NH = int(os.environ.get("CVCL_NH", "16"))  # half k-tiles (64 rows each)
NT_F = (N_LOC - NH * 64) // P  # full k-tiles of 128 rows
N_F = NT_F * P  # rows in full k-tiles
CHUNK = int(os.environ.get("CVCL_CHUNK", "8"))  # k-tiles per DMA
BUFS = int(os.environ.get("CVCL_BUFS", "6"))  # tile-pool depth
ALPHA = 9.0
EPS = 2.220446049250313e-16

# matmul input mode: "bf16" (ACT/DVE cast) or "dmacast" (SWDGE casts in-flight)
MM_MODE = os.environ.get("CVCL_MM_MODE", "bf16")
# input DMA queue assignment: "hwsplit" (v1 sync / v2 scalar, both HWDGE),
# "split" (v1 sync / v2 gpsimd), "sync" (both on sync)
DMA_Q = os.environ.get("CVCL_DMA_Q", "hwsplit")
# descending chunk sizes at the end to shrink the post-DMA tail
TAIL = os.environ.get("CVCL_TAIL", "1") == "1"

_BUILD_CACHE = {}
LAST_RESULT = None  # BassKernelResults of the most recent run (for test.py)


def _install_axon_hooks_shim():
    """bass_utils' trace path imports antenv.axon_hooks, which this image
    lacks. Provide it, wiring the ctypes NTFF hook from trn_boot when the
    axon .so supports it. Harmless no-op when tracing is off."""
    import sys
    import types

    try:
        from antenv import axon_hooks  # noqa: F401

        return
    except ImportError:
        pass
    try:
        import antenv
    except ImportError:
        return
    mod = types.ModuleType("antenv.axon_hooks")
    mod._hook = None
    mod._resolved = False

    def set_axon_ntff_profile_hook(h):
        mod._hook = h
        mod._resolved = True

    def get_axon_ntff_profile_hook():
        # lazy: only touch the axon .so when tracing is actually requested
        if not mod._resolved:
            mod._resolved = True
            try:
                from trn_agent_boot.trn_boot import _ntff_profile_via_ctypes

                so_path = "/opt/axon/libaxon_pjrt.so"
                if os.path.exists(so_path):
                    mod._hook = _ntff_profile_via_ctypes(so_path)
            except Exception:
                mod._hook = None
        return mod._hook

    mod.set_axon_ntff_profile_hook = set_axon_ntff_profile_hook
    mod.get_axon_ntff_profile_hook = get_axon_ntff_profile_hook
    sys.modules["antenv.axon_hooks"] = mod
    antenv.axon_hooks = mod


try:
    _install_axon_hooks_shim()
except Exception:
    pass


def _work_schedule():
    """List of ("f"|"h", k-tile offset, k-tile count) DMA-chunk work items.
    Full-tile sizes: small head chunks start the PE early (HAM warmup +
    pipeline prime); a short descending tail leaves minimal compute after
    the last byte lands; CHUNK-sized chunks fill the middle.  Half-tile
    chunks (even DMA engines only) are interleaved mid-stream so they
    never gate the tail."""
    if TAIL:
        head = [s for s in (1, 2, 4) if s < CHUNK]
        tail = [s for s in (4, 2, 1, 1) if s < CHUNK]
    else:
        head, tail = [], []
    rem = NT_F - sum(head) - sum(tail)
    assert rem > 0
    mid = [CHUNK] * (rem // CHUNK)
    if rem % CHUNK:
        mid = [rem % CHUNK] + mid
    sizes = head + mid + tail
    assert sum(sizes) == NT_F

    items = []
    off = 0
    for csz in sizes:
        items.append(("f", off, csz))
        off += csz
    # interleave half-tile chunks (CHUNK halves each) after ~1/3 of the
    # full chunks
    hoff = 0
    pos = len(sizes) // 3
    while hoff < NH:
        hsz = min(CHUNK, NH - hoff)
        items.insert(pos, ("h", hoff, hsz))
        hoff += hsz
        pos += 2
    return items


def _build(mode):
    key = (mode, DMA_Q, CHUNK, TAIL)
    if key in _BUILD_CACHE:
        return _BUILD_CACHE[key]

    nc = bacc.Bacc(
        "TRN2", target_bir_lowering=False, debug=False, num_devices=NCORES
    )
    # host-rearranged, partition-major: row t*128+p of the shard lives at
    # v[p, t*K : (t+1)*K] -- each partition's stream is fully contiguous
    v1 = nc.dram_tensor("v1", [P, NT_F * K], mybir.dt.float32, kind="ExternalInput")
    v2 = nc.dram_tensor("v2", [P, NT_F * K], mybir.dt.float32, kind="ExternalInput")
    if NH:
        v1h = nc.dram_tensor(
            "v1h", [64, NH * K], mybir.dt.float32, kind="ExternalInput"
        )
        v2h = nc.dram_tensor(
            "v2h", [64, NH * K], mybir.dt.float32, kind="ExternalInput"
        )
    # [p, c*K+n] holds joint row c*128+p, col n -- per-partition contiguous
    # write, reassembled on host
    out = nc.dram_tensor(
        "partial", [P, 2 * K], mybir.dt.float32, kind="ExternalOutput"
    )

    v1r = v1.ap()
    v2r = v2.ap()
    out_ap = out.ap()

    with tile.TileContext(nc) as tc:
        with (
            tc.tile_pool(name="io", bufs=BUFS) as io_pool,
            tc.tile_pool(name="cv", bufs=BUFS) as cv_pool,
            tc.tile_pool(name="acc", bufs=1, space="PSUM") as psum_pool,
            tc.tile_pool(name="res", bufs=1) as res_pool,
        ):
            # one PSUM bank per 128-row chunk of the [256, 256] output
            ps0 = psum_pool.tile([P, K], mybir.dt.float32)
            ps1 = psum_pool.tile([P, K], mybir.dt.float32)

            items = _work_schedule()
            n_ktiles = NT_F + NH
            q1 = {"hwsplit": nc.sync, "split": nc.sync, "sync": nc.sync}[DMA_Q]
            q2 = {"hwsplit": nc.scalar, "split": nc.gpsimd, "sync": nc.sync}[
                DMA_Q
            ]

            emitted = 0
            for kind, off, csz in items:
                sl = slice(off * K, (off + csz) * K)
                rows = P if kind == "f" else 64
                s1 = v1r[:, sl] if kind == "f" else v1h.ap()[:, sl]
                s2 = v2r[:, sl] if kind == "f" else v2h.ap()[:, sl]
                t1 = io_pool.tile([P, CHUNK * K], mybir.dt.float32, tag="r1")
                t2 = io_pool.tile([P, CHUNK * K], mybir.dt.float32, tag="r2")
                q1.dma_start(t1[0:rows, 0 : csz * K], s1)
                q2.dma_start(t2[0:rows, 0 : csz * K], s2)
                m1 = cv_pool.tile([P, CHUNK * K], mybir.dt.bfloat16, tag="c1")
                m2 = cv_pool.tile([P, CHUNK * K], mybir.dt.bfloat16, tag="c2")
                nc.scalar.copy(m1[0:rows, 0 : csz * K], t1[0:rows, 0 : csz * K])
                nc.vector.tensor_copy(
                    m2[0:rows, 0 : csz * K], t2[0:rows, 0 : csz * K]
                )
                for j in range(csz):
                    first = emitted == 0
                    last = emitted == n_ktiles - 1
                    lhs_a = m1[0:rows, j * K : j * K + 128]
                    lhs_b = m1[0:rows, j * K + 128 : (j + 1) * K]
                    rhs = m2[0:rows, j * K : (j + 1) * K]
                    nc.tensor.matmul(
                        ps0[:], lhs_a, rhs, start=first, stop=last
                    )
                    nc.tensor.matmul(
                        ps1[:], lhs_b, rhs, start=first, stop=last
                    )
                    emitted += 1
            assert emitted == n_ktiles

            res = res_pool.tile([P, 2 * K], mybir.dt.float32)
            # parallel PSUM drains (ACT and DVE may touch different banks),
            # each followed by its own 128 KB output DMA on that engine's
            # queue -- both transfers and completion receipts overlap
            nc.scalar.copy(res[:, 0:K], ps0[:])
            nc.scalar.dma_start(out_ap[:, 0:K], res[:, 0:K])
            nc.vector.tensor_copy(res[:, K : 2 * K], ps1[:])
            nc.sync.dma_start(out_ap[:, K : 2 * K], res[:, K : 2 * K])

    nc.compile()
    _BUILD_CACHE[key] = nc
    return nc


def _rearrange_shards(v):
    """[N_FULL, K] -> full part [NCORES, P, NT_F*K] + half part
    [NCORES, 64, NH*K]: core c, partition p holds rows {t*128+p} of the
    first N_F rows of its shard; the last NH*64 rows sit on partitions
    0-63 as half-height k-tiles.  Each partition's stream is contiguous."""
    x = v.reshape(NCORES, N_LOC, K)
    full = np.ascontiguousarray(
        x[:, :N_F]
        .reshape(NCORES, NT_F, P, K)
        .transpose(0, 2, 1, 3)
        .reshape(NCORES, P, NT_F * K)
    )
    if not NH:
        return full, None
    half = np.ascontiguousarray(
        x[:, N_F:]
        .reshape(NCORES, NH, 64, K)
        .transpose(0, 2, 1, 3)
        .reshape(NCORES, 64, NH * K)
    )
    return full, half


def kernel(latent_view_1, latent_view_2):
    global LAST_RESULT
    v1 = np.asarray(latent_view_1, dtype=np.float32)
    v2 = np.asarray(latent_view_2, dtype=np.float32)
    assert v1.shape == (N_FULL, K) and v2.shape == (N_FULL, K)

    nc = _build(MM_MODE)
    v1s, v1hs = _rearrange_shards(v1)
    v2s, v2hs = _rearrange_shards(v2)
    in_maps = [{"v1": v1s[c], "v2": v2s[c]} for c in range(NCORES)]
    if NH:
        for c in range(NCORES):
            in_maps[c]["v1h"] = v1hs[c]
            in_maps[c]["v2h"] = v2hs[c]
    LAST_RESULT = bass_utils.run_bass_kernel_spmd(
        nc, in_maps, core_ids=list(range(NCORES))
    )

    # host epilogue in float64 on the tiny [256, 256] joint
    p_i_j = np.zeros((K, K), dtype=np.float64)
    for r in LAST_RESULT.results:
        p_i_j += np.asarray(r["partial"], dtype=np.float64)
    p_i_j = (p_i_j + p_i_j.T) / 2.0
    p_i_j = p_i_j / p_i_j.sum()
    p_i = p_i_j.sum(axis=1, keepdims=True)
    p_j = p_i_j.sum(axis=0, keepdims=True)
    p_i_j = np.maximum(p_i_j, EPS)
    p_i = np.maximum(p_i, EPS)
    p_j = np.maximum(p_j, EPS)
    loss = -(
        p_i_j
        * (
            np.log(p_i_j)
            - (ALPHA + 1.0) * np.log(p_j)
            - (ALPHA + 1.0) * np.log(p_i)
        )
    ).sum()
    return np.array(loss, dtype=np.float32)


# revision 18
# speedup vs baseline: 1.2486x; 1.0216x over previous
"""CrossViewContrastiveLoss Trainium2 kernel.

loss = f(v1^T @ v2) where v1, v2 are [131072, 256] fp32 and f is a cheap
normalize/log epilogue on the [256, 256] joint matrix.

Strategy (data-parallel over N across 8 cores):
  - host rearranges each core's [16384, 256] row-shard to partition-major
    [128, 32768] so every DMA descriptor is a large contiguous read
    (8 KiB vs 1 KiB) -- keeps the 16 SDMA engines near the ~358 GB/s
    per-core HBM cap (~94 us stream floor for 32 MiB/core).
  - two HWDGE queues (v1 on sync, v2 on scalar), 6-deep tile buffering so
    the descriptor rings never run dry; small head chunks (1,2,4 k-tiles)
    prime the PE / HAM clock-gate early; a descending tail (4,2,1,1)
    leaves almost no compute after the last byte lands.
  - fp32 tiles are cast to bf16 (v1 on ACT, v2 on DVE -- both have 2x
    slack vs the stream) because the PE needs a compute-op-rounded input;
    matmuls accumulate the [256,256] joint into two PSUM banks.
  - PSUM drains run in parallel (ACT + DVE), each followed by its own
    128 KB output DMA on a separate queue.
  - host sums the eight 256x256 partials in float64 and runs the epilogue
    (65536 elements -- negligible next to 256 MiB of streaming).
"""

import os

import numpy as np

import concourse.bacc as bacc
import concourse.bass as bass
import concourse.mybir as mybir
import concourse.tile as tile
from concourse import bass_utils

N_FULL = 131072
K = 256
NCORES = 8
N_LOC = N_FULL // NCORES  # 16384 rows per core
P = 128
# Optional rebalance for the slow DMA engine 15 (serves partitions
# {92-95,124-127}): pack the last NH*64 rows of each shard as half-height
# k-tiles on partitions 0-63 (even engines only).  Measured neutral-to-
# negative against run-to-run noise, so default off.
NH = int(os.environ.get("CVCL_NH", "0"))  # half k-tiles (64 rows each)
NT_F = (N_LOC - NH * 64) // P  # full k-tiles of 128 rows
N_F = NT_F * P  # rows in full k-tiles
CHUNK = int(os.environ.get("CVCL_CHUNK", "8"))  # k-tiles per DMA
BUFS = int(os.environ.get("CVCL_BUFS", "6"))  # tile-pool depth
ALPHA = 9.0
EPS = 2.220446049250313e-16

# matmul input mode: "bf16" (ACT/DVE cast) or "dmacast" (SWDGE casts in-flight)
MM_MODE = os.environ.get("CVCL_MM_MODE", "bf16")
# input DMA queue assignment: "hwsplit" (v1 sync / v2 scalar, both HWDGE),
# "split" (v1 sync / v2 gpsimd), "sync" (both on sync)
DMA_Q = os.environ.get("CVCL_DMA_Q", "hwsplit")
# descending chunk sizes at the end to shrink the post-DMA tail
TAIL = os.environ.get("CVCL_TAIL", "1") == "1"

_BUILD_CACHE = {}
LAST_RESULT = None  # BassKernelResults of the most recent run (for test.py)


def _install_axon_hooks_shim():
    """bass_utils' trace path imports antenv.axon_hooks, which this image
    lacks. Provide it, wiring the ctypes NTFF hook from trn_boot when the
    axon .so supports it. Harmless no-op when tracing is off."""
    import sys
    import types

    try:
        from antenv import axon_hooks  # noqa: F401

        return
    except ImportError:
        pass
    try:
        import antenv
    except ImportError:
        return
    mod = types.ModuleType("antenv.axon_hooks")
    mod._hook = None
    mod._resolved = False

    def set_axon_ntff_profile_hook(h):
        mod._hook = h
        mod._resolved = True

    def get_axon_ntff_profile_hook():
        # lazy: only touch the axon .so when tracing is actually requested
        if not mod._resolved:
            mod._resolved = True
            try:
                from trn_agent_boot.trn_boot import _ntff_profile_via_ctypes

                so_path = "/opt/axon/libaxon_pjrt.so"
                if os.path.exists(so_path):
                    mod._hook = _ntff_profile_via_ctypes(so_path)
            except Exception:
                mod._hook = None
        return mod._hook

    mod.set_axon_ntff_profile_hook = set_axon_ntff_profile_hook
    mod.get_axon_ntff_profile_hook = get_axon_ntff_profile_hook
    sys.modules["antenv.axon_hooks"] = mod
    antenv.axon_hooks = mod


try:
    _install_axon_hooks_shim()
except Exception:
    pass


def _work_schedule():
    """List of ("f"|"h", k-tile offset, k-tile count) DMA-chunk work items.
    Full-tile sizes: small head chunks start the PE early (HAM warmup +
    pipeline prime); a short descending tail leaves minimal compute after
    the last byte lands; CHUNK-sized chunks fill the middle.  Half-tile
    chunks (even DMA engines only) are interleaved mid-stream so they
    never gate the tail."""
    if TAIL:
        head = [s for s in (1, 2, 4) if s < CHUNK]
        tail = [s for s in (4, 2, 1, 1) if s < CHUNK]
    else:
        head, tail = [], []
    rem = NT_F - sum(head) - sum(tail)
    assert rem > 0
    mid = [CHUNK] * (rem // CHUNK)
    if rem % CHUNK:
        mid = [rem % CHUNK] + mid
    sizes = head + mid + tail
    assert sum(sizes) == NT_F

    items = []
    off = 0
    for csz in sizes:
        items.append(("f", off, csz))
        off += csz
    # interleave half-tile chunks (CHUNK halves each) after ~1/3 of the
    # full chunks
    hoff = 0
    pos = len(sizes) // 3
    while hoff < NH:
        hsz = min(CHUNK, NH - hoff)
        items.insert(pos, ("h", hoff, hsz))
        hoff += hsz
        pos += 2
    return items


def _build(mode):
    key = (mode, DMA_Q, CHUNK, TAIL)
    if key in _BUILD_CACHE:
        return _BUILD_CACHE[key]

    nc = bacc.Bacc(
        "TRN2", target_bir_lowering=False, debug=False, num_devices=NCORES
    )
    # host-rearranged, partition-major: row t*128+p of the shard lives at
    # v[p, t*K : (t+1)*K] -- each partition's stream is fully contiguous
    v1 = nc.dram_tensor("v1", [P, NT_F * K], mybir.dt.float32, kind="ExternalInput")
    v2 = nc.dram_tensor("v2", [P, NT_F * K], mybir.dt.float32, kind="ExternalInput")
    if NH:
        v1h = nc.dram_tensor(
            "v1h", [64, NH * K], mybir.dt.float32, kind="ExternalInput"
        )
        v2h = nc.dram_tensor(
            "v2h", [64, NH * K], mybir.dt.float32, kind="ExternalInput"
        )
    # [p, c*K+n] holds joint row c*128+p, col n -- per-partition contiguous
    # write, reassembled on host
    out = nc.dram_tensor(
        "partial", [P, 2 * K], mybir.dt.float32, kind="ExternalOutput"
    )

    v1r = v1.ap()
    v2r = v2.ap()
    out_ap = out.ap()

    with tile.TileContext(nc) as tc:
        with (
            tc.tile_pool(name="io", bufs=BUFS) as io_pool,
            tc.tile_pool(name="cv", bufs=BUFS) as cv_pool,
            tc.tile_pool(name="acc", bufs=1, space="PSUM") as psum_pool,
            tc.tile_pool(name="res", bufs=1) as res_pool,
        ):
            # one PSUM bank per 128-row chunk of the [256, 256] output
            ps0 = psum_pool.tile([P, K], mybir.dt.float32)
            ps1 = psum_pool.tile([P, K], mybir.dt.float32)

            items = _work_schedule()
            n_ktiles = NT_F + NH
            q1 = {"hwsplit": nc.sync, "split": nc.sync, "sync": nc.sync}[DMA_Q]
            q2 = {"hwsplit": nc.scalar, "split": nc.gpsimd, "sync": nc.sync}[
                DMA_Q
            ]

            emitted = 0
            for kind, off, csz in items:
                sl = slice(off * K, (off + csz) * K)
                rows = P if kind == "f" else 64
                s1 = v1r[:, sl] if kind == "f" else v1h.ap()[:, sl]
                s2 = v2r[:, sl] if kind == "f" else v2h.ap()[:, sl]
                t1 = io_pool.tile([P, CHUNK * K], mybir.dt.float32, tag="r1")
                t2 = io_pool.tile([P, CHUNK * K], mybir.dt.float32, tag="r2")
                q1.dma_start(t1[0:rows, 0 : csz * K], s1)
                q2.dma_start(t2[0:rows, 0 : csz * K], s2)
                m1 = cv_pool.tile([P, CHUNK * K], mybir.dt.bfloat16, tag="c1")
                m2 = cv_pool.tile([P, CHUNK * K], mybir.dt.bfloat16, tag="c2")
                nc.scalar.copy(m1[0:rows, 0 : csz * K], t1[0:rows, 0 : csz * K])
                nc.vector.tensor_copy(
                    m2[0:rows, 0 : csz * K], t2[0:rows, 0 : csz * K]
                )
                for j in range(csz):
                    first = emitted == 0
                    last = emitted == n_ktiles - 1
                    lhs_a = m1[0:rows, j * K : j * K + 128]
                    lhs_b = m1[0:rows, j * K + 128 : (j + 1) * K]
                    rhs = m2[0:rows, j * K : (j + 1) * K]
                    nc.tensor.matmul(
                        ps0[:], lhs_a, rhs, start=first, stop=last
                    )
                    nc.tensor.matmul(
                        ps1[:], lhs_b, rhs, start=first, stop=last
                    )
                    emitted += 1
            assert emitted == n_ktiles

            res = res_pool.tile([P, 2 * K], mybir.dt.float32)
            # parallel PSUM drains (ACT and DVE may touch different banks),
            # each followed by its own 128 KB output DMA on that engine's
            # queue -- both transfers and completion receipts overlap
            nc.scalar.copy(res[:, 0:K], ps0[:])
            nc.scalar.dma_start(out_ap[:, 0:K], res[:, 0:K])
            nc.vector.tensor_copy(res[:, K : 2 * K], ps1[:])
            nc.sync.dma_start(out_ap[:, K : 2 * K], res[:, K : 2 * K])

    nc.compile()
    _BUILD_CACHE[key] = nc
    return nc


def _rearrange_shards(v):
    """[N_FULL, K] -> full part [NCORES, P, NT_F*K] + half part
    [NCORES, 64, NH*K]: core c, partition p holds rows {t*128+p} of the
    first N_F rows of its shard; the last NH*64 rows sit on partitions
    0-63 as half-height k-tiles.  Each partition's stream is contiguous."""
    x = v.reshape(NCORES, N_LOC, K)
    full = np.ascontiguousarray(
        x[:, :N_F]
        .reshape(NCORES, NT_F, P, K)
        .transpose(0, 2, 1, 3)
        .reshape(NCORES, P, NT_F * K)
    )
    if not NH:
        return full, None
    half = np.ascontiguousarray(
        x[:, N_F:]
        .reshape(NCORES, NH, 64, K)
        .transpose(0, 2, 1, 3)
        .reshape(NCORES, 64, NH * K)
    )
    return full, half


def kernel(latent_view_1, latent_view_2):
    global LAST_RESULT
    v1 = np.asarray(latent_view_1, dtype=np.float32)
    v2 = np.asarray(latent_view_2, dtype=np.float32)
    assert v1.shape == (N_FULL, K) and v2.shape == (N_FULL, K)

    nc = _build(MM_MODE)
    v1s, v1hs = _rearrange_shards(v1)
    v2s, v2hs = _rearrange_shards(v2)
    in_maps = [{"v1": v1s[c], "v2": v2s[c]} for c in range(NCORES)]
    if NH:
        for c in range(NCORES):
            in_maps[c]["v1h"] = v1hs[c]
            in_maps[c]["v2h"] = v2hs[c]
    LAST_RESULT = bass_utils.run_bass_kernel_spmd(
        nc, in_maps, core_ids=list(range(NCORES))
    )

    # host epilogue in float64 on the tiny [256, 256] joint;
    # device layout: joint row c*128+p lives at partial[p, c*K:(c+1)*K]
    p_i_j = np.zeros((K, K), dtype=np.float64)
    for r in LAST_RESULT.results:
        buf = np.asarray(r["partial"], dtype=np.float64)
        p_i_j[:P] += buf[:, :K]
        p_i_j[P:] += buf[:, K:]
    p_i_j = (p_i_j + p_i_j.T) / 2.0
    p_i_j = p_i_j / p_i_j.sum()
    p_i = p_i_j.sum(axis=1, keepdims=True)
    p_j = p_i_j.sum(axis=0, keepdims=True)
    p_i_j = np.maximum(p_i_j, EPS)
    p_i = np.maximum(p_i, EPS)
    p_j = np.maximum(p_j, EPS)
    loss = -(
        p_i_j
        * (
            np.log(p_i_j)
            - (ALPHA + 1.0) * np.log(p_j)
            - (ALPHA + 1.0) * np.log(p_i)
        )
    ).sum()
    return np.array(loss, dtype=np.float32)


# revision 19
# speedup vs baseline: 1.2513x; 1.0022x over previous
"""CrossViewContrastiveLoss Trainium2 kernel.

loss = f(v1^T @ v2) where v1, v2 are [131072, 256] fp32 and f is a cheap
normalize/log epilogue on the [256, 256] joint matrix.

Strategy (data-parallel over N across 8 cores):
  - host rearranges each core's [16384, 256] row-shard to partition-major
    [128, 32768] so every DMA descriptor is a large contiguous read
    (8 KiB vs 1 KiB) -- keeps the 16 SDMA engines near the ~358 GB/s
    per-core HBM cap (~94 us stream floor for 32 MiB/core).
  - two HWDGE queues (v1 on sync, v2 on scalar), 6-deep tile buffering so
    the descriptor rings never run dry; small head chunks (1,2,4 k-tiles)
    prime the PE / HAM clock-gate early; a descending tail (4,2,1,1)
    leaves almost no compute after the last byte lands.
  - fp32 tiles are cast to bf16 (v1 on ACT, v2 on DVE -- both have 2x
    slack vs the stream) because the PE needs a compute-op-rounded input;
    matmuls accumulate the [256,256] joint into two PSUM banks.
  - PSUM drains run in parallel (ACT + DVE), each followed by its own
    128 KB output DMA on a separate queue.
  - host sums the eight 256x256 partials in float64 and runs the epilogue
    (65536 elements -- negligible next to 256 MiB of streaming).
"""

import os

import numpy as np

import concourse.bacc as bacc
import concourse.bass as bass
import concourse.mybir as mybir
import concourse.tile as tile
from concourse import bass_utils

N_FULL = 131072
K = 256
NCORES = 8
N_LOC = N_FULL // NCORES  # 16384 rows per core
P = 128
# Optional rebalance for the slow DMA engine 15 (serves partitions
# {92-95,124-127}): pack the last NH*64 rows of each shard as half-height
# k-tiles on partitions 0-63 (even engines only).  Measured neutral-to-
# negative against run-to-run noise, so default off.
NH = int(os.environ.get("CVCL_NH", "0"))  # half k-tiles (64 rows each)
NT_F = (N_LOC - NH * 64) // P  # full k-tiles of 128 rows
N_F = NT_F * P  # rows in full k-tiles
CHUNK = int(os.environ.get("CVCL_CHUNK", "8"))  # k-tiles per DMA
BUFS = int(os.environ.get("CVCL_BUFS", "6"))  # tile-pool depth
ALPHA = 9.0
EPS = 2.220446049250313e-16

# matmul input mode: "bf16" (ACT/DVE cast) or "dmacast" (SWDGE casts in-flight)
MM_MODE = os.environ.get("CVCL_MM_MODE", "bf16")
# input DMA queue assignment: "hwsplit" (v1 sync / v2 scalar, both HWDGE),
# "split" (v1 sync / v2 gpsimd), "sync" (both on sync)
DMA_Q = os.environ.get("CVCL_DMA_Q", "hwsplit")
# descending chunk sizes at the end to shrink the post-DMA tail
TAIL = os.environ.get("CVCL_TAIL", "1") == "1"

_BUILD_CACHE = {}
LAST_RESULT = None  # BassKernelResults of the most recent run (for test.py)


def _install_axon_hooks_shim():
    """bass_utils' trace path imports antenv.axon_hooks, which this image
    lacks. Provide it, wiring the ctypes NTFF hook from trn_boot when the
    axon .so supports it. Harmless no-op when tracing is off."""
    import sys
    import types

    try:
        from antenv import axon_hooks  # noqa: F401

        return
    except ImportError:
        pass
    try:
        import antenv
    except ImportError:
        return
    mod = types.ModuleType("antenv.axon_hooks")
    mod._hook = None
    mod._resolved = False

    def set_axon_ntff_profile_hook(h):
        mod._hook = h
        mod._resolved = True

    def get_axon_ntff_profile_hook():
        # lazy: only touch the axon .so when tracing is actually requested
        if not mod._resolved:
            mod._resolved = True
            try:
                from trn_agent_boot.trn_boot import _ntff_profile_via_ctypes

                so_path = "/opt/axon/libaxon_pjrt.so"
                if os.path.exists(so_path):
                    mod._hook = _ntff_profile_via_ctypes(so_path)
            except Exception:
                mod._hook = None
        return mod._hook

    mod.set_axon_ntff_profile_hook = set_axon_ntff_profile_hook
    mod.get_axon_ntff_profile_hook = get_axon_ntff_profile_hook
    sys.modules["antenv.axon_hooks"] = mod
    antenv.axon_hooks = mod


try:
    _install_axon_hooks_shim()
except Exception:
    pass


def _work_schedule():
    """List of ("f"|"h", k-tile offset, k-tile count) DMA-chunk work items.
    Full-tile sizes: small head chunks start the PE early (HAM warmup +
    pipeline prime); a short descending tail leaves minimal compute after
    the last byte lands; CHUNK-sized chunks fill the middle.  Half-tile
    chunks (even DMA engines only) are interleaved mid-stream so they
    never gate the tail."""
    if TAIL:
        head = [s for s in (1, 2, 4) if s < CHUNK]
        tail = [s for s in (4, 2, 1, 1) if s < CHUNK]
    else:
        head, tail = [], []
    rem = NT_F - sum(head) - sum(tail)
    assert rem > 0
    mid = [CHUNK] * (rem // CHUNK)
    if rem % CHUNK:
        mid = [rem % CHUNK] + mid
    sizes = head + mid + tail
    assert sum(sizes) == NT_F

    items = []
    off = 0
    for csz in sizes:
        items.append(("f", off, csz))
        off += csz
    # interleave half-tile chunks (CHUNK halves each) after ~1/3 of the
    # full chunks
    hoff = 0
    pos = len(sizes) // 3
    while hoff < NH:
        hsz = min(CHUNK, NH - hoff)
        items.insert(pos, ("h", hoff, hsz))
        hoff += hsz
        pos += 2
    return items


def _build(mode):
    key = (mode, DMA_Q, CHUNK, TAIL)
    if key in _BUILD_CACHE:
        return _BUILD_CACHE[key]

    nc = bacc.Bacc(
        "TRN2", target_bir_lowering=False, debug=False, num_devices=NCORES
    )
    # host-rearranged, partition-major: row t*128+p of the shard lives at
    # v[p, t*K : (t+1)*K] -- each partition's stream is fully contiguous
    v1 = nc.dram_tensor("v1", [P, NT_F * K], mybir.dt.float32, kind="ExternalInput")
    v2 = nc.dram_tensor("v2", [P, NT_F * K], mybir.dt.float32, kind="ExternalInput")
    if NH:
        v1h = nc.dram_tensor(
            "v1h", [64, NH * K], mybir.dt.float32, kind="ExternalInput"
        )
        v2h = nc.dram_tensor(
            "v2h", [64, NH * K], mybir.dt.float32, kind="ExternalInput"
        )
    # [p, c*K+n] holds joint row c*128+p, col n -- per-partition contiguous
    # write, reassembled on host
    out = nc.dram_tensor(
        "partial", [P, 2 * K], mybir.dt.float32, kind="ExternalOutput"
    )

    v1r = v1.ap()
    v2r = v2.ap()
    out_ap = out.ap()

    with tile.TileContext(nc) as tc:
        with (
            tc.tile_pool(name="io", bufs=BUFS) as io_pool,
            tc.tile_pool(name="cv", bufs=BUFS) as cv_pool,
            tc.tile_pool(name="acc", bufs=1, space="PSUM") as psum_pool,
            tc.tile_pool(name="res", bufs=1) as res_pool,
        ):
            # one PSUM bank per 128-row chunk of the [256, 256] output
            ps0 = psum_pool.tile([P, K], mybir.dt.float32)
            ps1 = psum_pool.tile([P, K], mybir.dt.float32)

            items = _work_schedule()
            n_ktiles = NT_F + NH
            q1 = {"hwsplit": nc.sync, "split": nc.sync, "sync": nc.sync}[DMA_Q]
            q2 = {"hwsplit": nc.scalar, "split": nc.gpsimd, "sync": nc.sync}[
                DMA_Q
            ]

            emitted = 0
            for kind, off, csz in items:
                sl = slice(off * K, (off + csz) * K)
                rows = P if kind == "f" else 64
                s1 = v1r[:, sl] if kind == "f" else v1h.ap()[:, sl]
                s2 = v2r[:, sl] if kind == "f" else v2h.ap()[:, sl]
                t1 = io_pool.tile([P, CHUNK * K], mybir.dt.float32, tag="r1")
                t2 = io_pool.tile([P, CHUNK * K], mybir.dt.float32, tag="r2")
                q1.dma_start(t1[0:rows, 0 : csz * K], s1)
                q2.dma_start(t2[0:rows, 0 : csz * K], s2)
                m1 = cv_pool.tile([P, CHUNK * K], mybir.dt.bfloat16, tag="c1")
                m2 = cv_pool.tile([P, CHUNK * K], mybir.dt.bfloat16, tag="c2")
                nc.scalar.copy(m1[0:rows, 0 : csz * K], t1[0:rows, 0 : csz * K])
                nc.vector.tensor_copy(
                    m2[0:rows, 0 : csz * K], t2[0:rows, 0 : csz * K]
                )
                for j in range(csz):
                    first = emitted == 0
                    last = emitted == n_ktiles - 1
                    lhs_a = m1[0:rows, j * K : j * K + 128]
                    lhs_b = m1[0:rows, j * K + 128 : (j + 1) * K]
                    rhs = m2[0:rows, j * K : (j + 1) * K]
                    nc.tensor.matmul(
                        ps0[:], lhs_a, rhs, start=first, stop=last
                    )
                    nc.tensor.matmul(
                        ps1[:], lhs_b, rhs, start=first, stop=last
                    )
                    emitted += 1
            assert emitted == n_ktiles

            res = res_pool.tile([P, 2 * K], mybir.dt.float32)
            # parallel PSUM drains (ACT and DVE may touch different banks),
            # each followed by its own 128 KB output DMA on that engine's
            # queue -- both transfers and completion receipts overlap
            nc.scalar.copy(res[:, 0:K], ps0[:])
            nc.scalar.dma_start(out_ap[:, 0:K], res[:, 0:K])
            nc.vector.tensor_copy(res[:, K : 2 * K], ps1[:])
            nc.sync.dma_start(out_ap[:, K : 2 * K], res[:, K : 2 * K])

    nc.compile()
    _BUILD_CACHE[key] = nc
    return nc


def _rearrange_shards(v):
    """[N_FULL, K] -> full part [NCORES, P, NT_F*K] + half part
    [NCORES, 64, NH*K]: core c, partition p holds rows {t*128+p} of the
    first N_F rows of its shard; the last NH*64 rows sit on partitions
    0-63 as half-height k-tiles.  Each partition's stream is contiguous."""
    x = v.reshape(NCORES, N_LOC, K)
    full = np.ascontiguousarray(
        x[:, :N_F]
        .reshape(NCORES, NT_F, P, K)
        .transpose(0, 2, 1, 3)
        .reshape(NCORES, P, NT_F * K)
    )
    if not NH:
        return full, None
    half = np.ascontiguousarray(
        x[:, N_F:]
        .reshape(NCORES, NH, 64, K)
        .transpose(0, 2, 1, 3)
        .reshape(NCORES, 64, NH * K)
    )
    return full, half


def kernel(latent_view_1, latent_view_2):
    global LAST_RESULT
    v1 = np.asarray(latent_view_1, dtype=np.float32)
    v2 = np.asarray(latent_view_2, dtype=np.float32)
    assert v1.shape == (N_FULL, K) and v2.shape == (N_FULL, K)

    nc = _build(MM_MODE)
    v1s, v1hs = _rearrange_shards(v1)
    v2s, v2hs = _rearrange_shards(v2)
    in_maps = [{"v1": v1s[c], "v2": v2s[c]} for c in range(NCORES)]
    if NH:
        for c in range(NCORES):
            in_maps[c]["v1h"] = v1hs[c]
            in_maps[c]["v2h"] = v2hs[c]
    try:
        LAST_RESULT = bass_utils.run_bass_kernel_spmd(
            nc, in_maps, core_ids=list(range(NCORES))
        )
    except Exception:
        # transient device/tunnel failures (e.g. NRT_EXEC_UNIT_UNRECOVERABLE
        # from a wedged core) usually clear on re-execution
        LAST_RESULT = bass_utils.run_bass_kernel_spmd(
            nc, in_maps, core_ids=list(range(NCORES))
        )

    # host epilogue in float64 on the tiny [256, 256] joint;
    # device layout: joint row c*128+p lives at partial[p, c*K:(c+1)*K]
    p_i_j = np.zeros((K, K), dtype=np.float64)
    for r in LAST_RESULT.results:
        buf = np.asarray(r["partial"], dtype=np.float64)
        p_i_j[:P] += buf[:, :K]
        p_i_j[P:] += buf[:, K:]
    p_i_j = (p_i_j + p_i_j.T) / 2.0
    p_i_j = p_i_j / p_i_j.sum()
    p_i = p_i_j.sum(axis=1, keepdims=True)
    p_j = p_i_j.sum(axis=0, keepdims=True)
    p_i_j = np.maximum(p_i_j, EPS)
    p_i = np.maximum(p_i, EPS)
    p_j = np.maximum(p_j, EPS)
    loss = -(
        p_i_j
        * (
            np.log(p_i_j)
            - (ALPHA + 1.0) * np.log(p_j)
            - (ALPHA + 1.0) * np.log(p_i)
        )
    ).sum()
    return np.array(loss, dtype=np.float32)
